# revision 2
# baseline (speedup 1.0000x reference)
"""Trainium2 Bass kernel v2 for nn_LossFunctions_86397562126683.

Pure data parallel over 8 cores (1024 batch rows each); per-core partial
scalar loss summed on host.

Key structure (per core, 8 chunks of 128 batch rows):
  - host ships fp16 tensors: d-major transposed ai/a_hat/o (gram operands),
    natural a_hat/mu/logvar, plus f32 mask in both layouts -> all SBUF
    resident, ~2 DMAs per tensor total
  - per 16-row block: fp16 PE grams with a rank-17 +BIG off-diagonal mask
    accumulated in, so a single segmented min-reduce extracts the per-row
    gram diagonal block (no mask multiply)
  - slot norms via ones-vector matmuls over fp16 squares, repartitioned to
    natural layout through a small DRAM bounce
  - SQ = ni + nh - 2G, D = sqrt(relu(SQ)); eps-packing M = D + eps*V lets
    one min-chain recover the matched-V sum: V = (minP - minT)/eps, so no
    equality-mask extraction pass exists
  - permutation totals: fp16 matmuls against 2+2+3 meet-in-the-middle
    one-hot tables (triples on rows 0-2; quads via pair-mins on rows (3,4)
    x (5,6) recombined through a selection matmul), segmented min-reduces
  - KL / entropy / reorder are batched into one exp/ln tail phase so the
    ACT function-table loads happen twice total instead of per chunk
"""

import itertools
import os
import sys

import numpy as np

sys.path.insert(0, "/opt/trn_rl_repo")

BATCH = 8192
N_CORES = 8
B = BATCH // N_CORES          # 1024 per core
S = 7
D = 128
NBH = B // 128                # 8 chunks of 128
BETA = 4.0
EPS = 0.01
BIG = 32768.0

_nc_cache = {}


def _build_tables():
    # MT: [49, 210] one-hots for ordered triples on rows 0-2 (35 subsets x 6)
    # MB: [49, 840] one-hots for ordered quads on rows 3-6 (35 subsets x 24)
    s_sets = list(itertools.combinations(range(S), 3))  # 35, lex order
    MT = np.zeros((49, 210), np.float32)
    MB = np.zeros((49, 840), np.float32)
    for si, sset in enumerate(s_sets):
        for k, perm in enumerate(itertools.permutations(sset)):
            for i, j in enumerate(perm):
                MT[i * 7 + j, si * 6 + k] = 1.0
        quad = tuple(sorted(set(range(S)) - set(sset)))
        for k, perm in enumerate(itertools.permutations(quad)):
            for i2, j in enumerate(perm):
                MB[(3 + i2) * 7 + j, si * 24 + k] = 1.0
    return MT, MB


def _expand_rows(T49, a):
    # feed rows are (i, a, j): row = i*16 + a*8 + j
    out = np.zeros((112, T49.shape[1]), np.float32)
    for i in range(7):
        for j in range(7):
            out[i * 16 + a * 8 + j] = T49[i * 7 + j]
    return out


def _build_device_tables():
    # 2+2+3 meet-in-the-middle:
    #   A  [112, 2, 256]: totals of ordered triples on rows 0-2 (35 x 6)
    #   C  [112, 2, 128]: totals of ordered pairs on rows (3,4) / (5,6)
    #                     cols = half*42 + pair*2 + order
    #   SEL [84, 2, 256]: quad-total = m21[half0, P1] + m21[half1, P2]
    #                     over the 6 pair-splits of each of 35 quads
    MT, _ = _build_tables()
    A = np.zeros((112, 2, 256), np.float16)
    for a in range(2):
        A[:, a, :210] = _expand_rows(MT, a)
    pairs = list(itertools.combinations(range(S), 2))
    pidx = {p: k for k, p in enumerate(pairs)}
    s_sets = list(itertools.combinations(range(S), 3))
    C = np.zeros((112, 2, 128), np.float16)
    for a in range(2):
        for half, (r0, r1) in enumerate(((3, 4), (5, 6))):
            for pp, (q0, q1) in enumerate(pairs):
                for o, (p0, p1) in enumerate(((q0, q1), (q1, q0))):
                    col = half * 42 + pp * 2 + o
                    C[r0 * 16 + a * 8 + p0, a, col] = 1.0
                    C[r1 * 16 + a * 8 + p1, a, col] = 1.0
    SEL = np.zeros((84, 2, 256), np.float16)
    for a in range(2):
        for q, sset in enumerate(s_sets):
            quad = tuple(sorted(set(range(S)) - set(sset)))
            for s, P1 in enumerate(itertools.combinations(quad, 2)):
                P2 = tuple(sorted(set(quad) - set(P1)))
                SEL[a * 42 + pidx[P1], a, q * 6 + s] = 1.0
                SEL[a * 42 + 21 + pidx[P2], a, q * 6 + s] = 1.0
    return A.reshape(112, 512), C.reshape(112, 256), SEL.reshape(84, 512)


def _build_umask():
    # rank-17 factors: U^T V = BIG * (1 - delta(g == g')) on the 112x112
    # gram block with p=(g,i), n=(g',j)
    U = np.zeros((17, 112), np.float16)
    V = np.zeros((17, 112), np.float16)
    U[0, :] = BIG
    V[0, :] = 1.0
    for g in range(16):
        U[1 + g, g * 7:(g + 1) * 7] = -BIG
        V[1 + g, g * 7:(g + 1) * 7] = 1.0
    return U, V


def build_bass():
    import contextlib

    import concourse.bacc as bacc
    import concourse.bass as bass
    import concourse.tile as tile
    from concourse import mybir
    from concourse.masks import make_identity
    from concourse import bass_isa

    f32 = mybir.dt.float32
    f16 = mybir.dt.float16
    Alu = mybir.AluOpType
    Act = mybir.ActivationFunctionType
    AX = mybir.AxisListType

    A_np, C_np, SEL_np = _build_device_tables()
    U_np, V_np = _build_umask()

    nc = bacc.Bacc(
        "TRN2",
        target_bir_lowering=False,
        debug=False,
        enable_asserts=False,
        num_devices=N_CORES,
    )

    ait_d = nc.dram_tensor("ai_t", [128, NBH, 128, 7], f16, kind="ExternalInput").ap()
    aht_d = nc.dram_tensor("ah_t", [128, NBH, 128, 7], f16, kind="ExternalInput").ap()
    ot_d = nc.dram_tensor("o_t", [128, NBH, 128, 7], f16, kind="ExternalInput").ap()
    ahn_d = nc.dram_tensor("ah_n", [128, NBH, 896], f16, kind="ExternalInput").ap()
    mun_d = nc.dram_tensor("mu_n", [128, NBH, 896], f16, kind="ExternalInput").ap()
    lvn_d = nc.dram_tensor("lv_n", [128, NBH, 896], f16, kind="ExternalInput").ap()
    mkn_d = nc.dram_tensor("mask_n", [128, NBH, 128], f32, kind="ExternalInput").ap()
    mkt_d = nc.dram_tensor("mask_t", [128, NBH, 128], f32, kind="ExternalInput").ap()
    gam_d = nc.dram_tensor("gamma", [S * D], f32, kind="ExternalInput").ap()
    out_d = nc.dram_tensor("out", [1, 1], f32, kind="ExternalOutput").ap()

    atab_d = nc.inline_tensor(A_np, "atab_c").ap()
    ctab_d = nc.inline_tensor(C_np, "ctab_c").ap()
    seltab_d = nc.inline_tensor(SEL_np, "seltab_c").ap()
    u17_d = nc.inline_tensor(U_np, "u17_c").ap()
    v17_d = nc.inline_tensor(V_np, "v17_c").ap()

    with tile.TileContext(nc) as tc:
        ctx = contextlib.ExitStack()
        with ctx:
            consts = ctx.enter_context(tc.tile_pool(name="consts", bufs=1))
            pres = ctx.enter_context(tc.tile_pool(name="res", bufs=1))
            state = ctx.enter_context(tc.tile_pool(name="state", bufs=1))
            pwork = ctx.enter_context(tc.tile_pool(name="work", bufs=2))
            psmall = ctx.enter_context(tc.tile_pool(name="small", bufs=2))
            pg = ctx.enter_context(tc.tile_pool(name="gpsum", bufs=1, space="PSUM"))
            ptp = ctx.enter_context(tc.tile_pool(name="tpsum", bufs=1, space="PSUM"))
            ptot = ctx.enter_context(tc.tile_pool(name="totpsum", bufs=1, space="PSUM"))
            pdram = ctx.enter_context(tc.tile_pool(name="dhop", bufs=2, space="DRAM"))

            # ---- constants -------------------------------------------------
            atab = consts.tile([112, 512], f16, tag="atab")
            ctab = consts.tile([112, 256], f16, tag="ctab")
            seltab = consts.tile([84, 512], f16, tag="seltab")
            u17 = consts.tile([17, 112], f16, tag="u17")
            v17 = consts.tile([17, 112], f16, tag="v17")
            identh = consts.tile([128, 128], f16, tag="identh")
            ones_h = consts.tile([128, 1], f16, tag="onesh")
            ones_f = consts.tile([128, 1], f32, tag="onesf")
            gam8 = consts.tile([128, 8], f32, tag="gam8")
            eps_c = consts.tile([128, 1], f32, tag="eps")
            nc.sync.dma_start(out=atab, in_=atab_d)
            nc.sync.dma_start(out=ctab, in_=ctab_d)
            nc.sync.dma_start(out=seltab, in_=seltab_d)
            nc.sync.dma_start(out=u17, in_=u17_d)
            nc.sync.dma_start(out=v17, in_=v17_d)
            make_identity(nc, identh)
            nc.vector.memset(ones_h, 1.0)
            nc.vector.memset(ones_f, 1.0)
            nc.vector.memset(eps_c, 1e-10)
            nc.gpsimd.memset(gam8, 0.0)
            gam_b = bass.AP(tensor=gam_d.tensor, offset=0, ap=[[0, 128], [1, 7]])
            nc.sync.dma_start(out=gam8[:, 0:7], in_=gam_b)

            # ---- resident inputs ------------------------------------------
            ait_s = pres.tile([128, NBH, 128, 7], f16, tag="ait")
            aht_s = pres.tile([128, NBH, 128, 7], f16, tag="aht")
            ot_s = pres.tile([128, NBH, 128, 7], f16, tag="ot")
            ahn_s = pres.tile([128, NBH, 896], f16, tag="ahn")
            mun_s = pres.tile([128, NBH, 896], f16, tag="mun")
            lvn_s = pres.tile([128, NBH, 896], f16, tag="lvn")
            mkn_s = pres.tile([128, NBH, 128], f32, tag="mkn")
            mkt_s = pres.tile([128, NBH, 128], f32, tag="mkt")
            for t_s, t_d in ((ait_s, ait_d), (aht_s, aht_d), (ot_s, ot_d),
                             (ahn_s, ahn_d), (mun_s, mun_d), (lvn_s, lvn_d),
                             (mkn_s, mkn_d), (mkt_s, mkt_d)):
                h = NBH // 2
                nc.sync.dma_start(out=t_s[:, 0:h], in_=t_d[:, 0:h])
                nc.sync.dma_start(out=t_s[:, h:NBH], in_=t_d[:, h:NBH])

            # ---- persistent accumulators ----------------------------------
            # ROWS rows: 0 KLA, 1 KLB, 2 KLC, 3 ENT, 4 REO
            ROWS = state.tile([128, 5, NBH], f32, tag="rows")
            MINS = state.tile([128, NBH, 4], f32, tag="mins")
            SM = state.tile([128, NBH], f32, tag="sm")

            nc.gpsimd.memset(ROWS, 0.0)

            for bh in range(NBH):
                # ==== r = a_hat / mask (transposed) ========================
                recT = psmall.tile([128, 128], f32, tag="recT")
                nc.vector.reciprocal(out=recT, in_=mkt_s[:, bh])
                rt = pwork.tile([128, 128, 7], f16, tag="rt")
                rec_bc = bass.AP(tensor=recT.tensor, offset=recT.offset,
                                 ap=[recT.ap[0], [1, 128], [0, 7]])
                nc.vector.tensor_tensor(out=rt, in0=aht_s[:, bh], in1=rec_bc,
                                        op=Alu.mult)

                # ==== norms: squares + partition all-reduce ================
                sqai = pwork.tile([128, 128, 7], f16, tag="sqai")
                nc.scalar.activation(out=sqai, in_=ait_s[:, bh], func=Act.Square)
                sqo = pwork.tile([128, 128, 7], f16, tag="sqo")
                nc.scalar.activation(out=sqo, in_=ot_s[:, bh], func=Act.Square)
                sqah = pwork.tile([128, 128, 7], f16, tag="sqah")
                nc.scalar.activation(out=sqah, in_=aht_s[:, bh], func=Act.Square)
                sqr = pwork.tile([128, 128, 7], f16, tag="sqr")
                nc.scalar.activation(out=sqr, in_=rt, func=Act.Square)

                r4 = pwork.tile([128, 4, 896], f16, tag="r4")
                for k, sqt in enumerate((sqai, sqah, sqo, sqr)):
                    sq_flat = bass.AP(tensor=sqt.tensor, offset=sqt.offset,
                                      ap=[sqt.ap[0], [1, 896]])
                    nc.gpsimd.partition_all_reduce(
                        r4[:, k, :], sq_flat, 128, bass_isa.ReduceOp.add)
                nhop = pdram.tile([4, 896], f16, tag="nhop")
                nc.sync.dma_start(out=nhop, in_=r4[0:1, :, :])
                nnat = psmall.tile([128, 4, 8], f16, tag="nnat")
                nc.gpsimd.memset(nnat, 0.0)
                nhop_b = bass.AP(
                    tensor=nhop.tensor, offset=nhop.offset,
                    ap=[[7, 128], [896, 4], [1, 7]],
                )
                nc.sync.dma_start(out=nnat[:, :, 0:7], in_=nhop_b)

                # ==== grams + BIG off-diag mask, min-extract ===============
                gext = pwork.tile([112, 8, 16], f16, tag="gext")
                nc.gpsimd.memset(gext, 0.0)
                gps = pg.tile([112, 8, 128], f32, tag="g", name="gps")
                for asg, (lt, rtt) in enumerate(((ait_s[:, bh], aht_s[:, bh]),
                                                 (ot_s[:, bh], rt))):
                    for mb in range(8):
                        lt_m = bass.AP(tensor=lt.tensor,
                                       offset=lt.offset + mb * 16 * 7,
                                       ap=[lt.ap[0], [7, 16], [1, 7]])
                        rt_m = bass.AP(tensor=rtt.tensor,
                                       offset=rtt.offset + mb * 16 * 7,
                                       ap=[rtt.ap[0], [7, 16], [1, 7]])
                        nc.tensor.matmul(gps[:, mb, 0:112], lt_m, rt_m,
                                         start=True, stop=False)
                        nc.tensor.matmul(gps[:, mb, 0:112], u17, v17,
                                         start=False, stop=True)
                    gview = bass.AP(tensor=gps.tensor, offset=gps.offset,
                                    ap=[gps.ap[0], [128, 8], [1, 7], [7, 16]])
                    nc.vector.tensor_reduce(
                        out=gext[:, :, asg * 8:asg * 8 + 7], in_=gview,
                        axis=AX.X, op=Alu.min,
                    )

                ghop = pdram.tile([112, 8, 16], f16, tag="ghop")
                nc.sync.dma_start(out=ghop, in_=gext)
                snat = pwork.tile([128, 7, 16], f16, tag="snat")
                # snat[b=(m,g), i, c] = ghop[(g,i), m, c]
                ghop_b = bass.AP(
                    tensor=ghop.tensor, offset=ghop.offset,
                    ap=[[16, 8], [7 * 8 * 16, 16], [8 * 16, 7], [1, 16]],
                )
                nc.sync.dma_start(out=snat, in_=ghop_b)

                # ==== natural-side SQ / D / V / M ==========================
                nL = bass.AP(tensor=nnat.tensor, offset=nnat.offset,
                             ap=[nnat.ap[0], [1, 7], [16, 2], [0, 8]])
                nR = bass.AP(tensor=nnat.tensor, offset=nnat.offset + 8,
                             ap=[nnat.ap[0], [0, 7], [16, 2], [1, 8]])
                nsum = pwork.tile([128, 7, 2, 8], f16, tag="nsum")
                nc.vector.tensor_tensor(out=nsum, in0=nL, in1=nR, op=Alu.add)
                SQ = pwork.tile([128, 7, 2, 8], f16, tag="sq")
                snat_v = bass.AP(tensor=snat.tensor, offset=snat.offset,
                                 ap=[snat.ap[0], [16, 7], [8, 2], [1, 8]])
                nc.vector.scalar_tensor_tensor(
                    out=SQ, in0=snat_v, scalar=-2.0, in1=nsum,
                    op0=Alu.mult, op1=Alu.add,
                )
                SQr = pwork.tile([128, 7, 2, 8], f16, tag="sqr2")
                nc.vector.tensor_scalar(out=SQr, in0=SQ, scalar1=0.0,
                                        scalar2=None, op0=Alu.max)
                Dm = pwork.tile([128, 7, 2, 8], f16, tag="dm")
                nc.scalar.activation(out=Dm, in_=SQr, func=Act.Sqrt)
                w1 = pwork.tile([128, 7, 8], f16, tag="w1")
                SQr_a1 = bass.AP(tensor=SQr.tensor, offset=SQr.offset + 8,
                                 ap=[SQr.ap[0], [16, 7], [1, 8]])
                gam_bc = bass.AP(tensor=gam8.tensor, offset=gam8.offset,
                                 ap=[gam8.ap[0], [0, 7], [1, 8]])
                nc.vector.scalar_tensor_tensor(
                    out=w1, in0=SQr_a1, scalar=0.5, in1=gam_bc,
                    op0=Alu.mult, op1=Alu.subtract,
                )
                V2 = pwork.tile([128, 7, 8], f16, tag="v2")
                nc.scalar.activation(out=V2, in_=w1, func=Act.Abs)
                MM = pwork.tile([128, 7, 2, 8], f16, tag="mm")
                SQr_a0 = bass.AP(tensor=SQr.tensor, offset=SQr.offset,
                                 ap=[SQr.ap[0], [16, 7], [1, 8]])
                Dm_a0 = bass.AP(tensor=Dm.tensor, offset=Dm.offset,
                                ap=[Dm.ap[0], [16, 7], [1, 8]])
                Dm_a1 = bass.AP(tensor=Dm.tensor, offset=Dm.offset + 8,
                                ap=[Dm.ap[0], [16, 7], [1, 8]])
                MM_a0 = bass.AP(tensor=MM.tensor, offset=MM.offset,
                                ap=[MM.ap[0], [16, 7], [1, 8]])
                MM_a1 = bass.AP(tensor=MM.tensor, offset=MM.offset + 8,
                                ap=[MM.ap[0], [16, 7], [1, 8]])
                nc.vector.scalar_tensor_tensor(
                    out=MM_a0, in0=SQr_a0, scalar=EPS, in1=Dm_a0,
                    op0=Alu.mult, op1=Alu.add,
                )
                nc.vector.scalar_tensor_tensor(
                    out=MM_a1, in0=V2, scalar=EPS, in1=Dm_a1,
                    op0=Alu.mult, op1=Alu.add,
                )

                # ==== feeds (transpose) ====================================
                tp = ptp.tile([112, 2, 128], f16, tag="tp", name="tp")
                MM_f = bass.AP(tensor=MM.tensor, offset=MM.offset,
                               ap=[MM.ap[0], [1, 112]])
                Dm_f = bass.AP(tensor=Dm.tensor, offset=Dm.offset,
                               ap=[Dm.ap[0], [1, 112]])
                nc.tensor.transpose(tp[:, 0, :], MM_f, identh)
                nc.tensor.transpose(tp[:, 1, :], Dm_f, identh)
                feeds = pwork.tile([112, 2, 128], f16, tag="feeds")
                nc.scalar.copy(out=feeds, in_=tp)

                # ==== totals + chains (P side then T side) =================
                tA = ptot.tile([128, 2, 256], f32, tag="tA", name="tA")
                ctot = ptot.tile([128, 2, 128], f32, tag="ct", name="ct")
                B210 = ptot.tile([128, 2, 256], f32, tag="b210", name="b210")
                for side in range(2):      # 0: packed M, 1: D
                    feed = feeds[:, side, :]
                    tA_flat = bass.AP(tensor=tA.tensor, offset=tA.offset,
                                      ap=[tA.ap[0], [1, 512]])
                    nc.tensor.matmul(tA_flat, feed, atab)
                    ct_flat = bass.AP(tensor=ctot.tensor, offset=ctot.offset,
                                      ap=[ctot.ap[0], [1, 256]])
                    nc.tensor.matmul(ct_flat, feed, ctab)
                    A35 = psmall.tile([128, 2, 35], f16, tag="a35")
                    tA_v = bass.AP(tensor=tA.tensor, offset=tA.offset,
                                   ap=[tA.ap[0], [256, 2], [6, 35], [1, 6]])
                    nc.vector.tensor_reduce(out=A35, in_=tA_v, axis=AX.X,
                                            op=Alu.min)
                    m21 = psmall.tile([128, 2, 42], f16, tag="m21")
                    ct_v = bass.AP(tensor=ctot.tensor, offset=ctot.offset,
                                   ap=[ctot.ap[0], [128, 2], [2, 42], [1, 2]])
                    nc.vector.tensor_reduce(out=m21, in_=ct_v, axis=AX.X,
                                            op=Alu.min)
                    m21f = bass.AP(tensor=m21.tensor, offset=m21.offset,
                                   ap=[m21.ap[0], [1, 84]])
                    m21T = ptp.tile([84, 128], f16, tag="m21T", name="m21T")
                    nc.tensor.transpose(m21T, m21f, identh)
                    m21sb = psmall.tile([84, 128], f16, tag="m21sb")
                    nc.scalar.copy(out=m21sb, in_=m21T)
                    B210_f = bass.AP(tensor=B210.tensor, offset=B210.offset,
                                     ap=[B210.ap[0], [1, 512]])
                    nc.tensor.matmul(B210_f, m21sb, seltab)
                    B35 = psmall.tile([128, 2, 35], f16, tag="b35")
                    B210_v = bass.AP(tensor=B210.tensor, offset=B210.offset,
                                     ap=[B210.ap[0], [256, 2], [6, 35], [1, 6]])
                    nc.vector.tensor_reduce(out=B35, in_=B210_v,
                                            axis=AX.X, op=Alu.min)
                    t35 = psmall.tile([128, 2, 35], f16, tag="t35")
                    nc.vector.tensor_tensor(out=t35, in0=A35, in1=B35,
                                            op=Alu.add)
                    mins_v = bass.AP(
                        tensor=MINS.tensor,
                        offset=MINS.offset + bh * 4 + side,
                        ap=[MINS.ap[0], [2, 2]],
                    )
                    nc.vector.tensor_reduce(out=mins_v, in_=t35, axis=AX.X,
                                            op=Alu.min)

            # ---- tail: exp/ln side terms in one table regime -------------
            ptail = ctx.enter_context(tc.tile_pool(name="tail", bufs=1))
            mun_f = bass.AP(tensor=mun_s.tensor, offset=mun_s.offset,
                            ap=[mun_s.ap[0], [1, NBH * 896]])
            lvn_f = bass.AP(tensor=lvn_s.tensor, offset=lvn_s.offset,
                            ap=[lvn_s.ap[0], [1, NBH * 896]])
            mkn_f = bass.AP(tensor=mkn_s.tensor, offset=mkn_s.offset,
                            ap=[mkn_s.ap[0], [1, NBH * 128]])
            tscr = ptail.tile([128, NBH * 896], f16, tag="tscr")
            nc.scalar.activation(out=tscr, in_=mun_f, func=Act.Square,
                                 accum_out=ROWS[:, 0, 0:1])
            tscr2 = ptail.tile([128, NBH * 896], f16, tag="tscr")
            nc.scalar.activation(out=tscr2, in_=lvn_f, func=Act.Exp,
                                 accum_out=ROWS[:, 1, 0:1])
            nc.vector.tensor_reduce(out=ROWS[:, 2, :], in_=lvn_s,
                                    axis=AX.X, op=Alu.add)
            lnm = ptail.tile([128, NBH * 128], f16, tag="lnm")
            nc.scalar.activation(out=lnm, in_=mkn_f, func=Act.Ln, bias=eps_c)
            jmt = ptail.tile([128, NBH * 128], f16, tag="jmt")
            nc.vector.scalar_tensor_tensor(
                out=jmt, in0=lnm, scalar=1.0, in1=mkn_f,
                op0=Alu.mult, op1=Alu.mult,
                accum_out=ROWS[:, 3, 0:1],
            )
            dift = ptail.tile([128, NBH, 6, 128], f16, tag="dift")
            nc.vector.tensor_tensor(
                out=dift,
                in0=bass.AP(tensor=ahn_s.tensor, offset=ahn_s.offset + 128,
                            ap=[ahn_s.ap[0], [896, NBH], [128, 6], [1, 128]]),
                in1=bass.AP(tensor=ahn_s.tensor, offset=ahn_s.offset,
                            ap=[ahn_s.ap[0], [896, NBH], [128, 6], [1, 128]]),
                op=Alu.subtract,
            )
            dsq = ptail.tile([128, NBH, 6, 128], f16, tag="tscr")
            nc.scalar.activation(out=dsq, in_=dift, func=Act.Square,
                                 accum_out=ROWS[:, 4, 0:1])
            nc.vector.tensor_reduce(out=SM, in_=mkn_s, axis=AX.X, op=Alu.add)

            # ---- final combine -------------------------------------------
            fin = state.tile([128, 5], f32, tag="fin")
            nc.vector.tensor_reduce(out=fin, in_=ROWS, axis=AX.X, op=Alu.add)

            # rec rows from MINS: dP = minP - minT per asg
            d0 = state.tile([128, NBH], f32, tag="d0")
            nc.vector.tensor_tensor(
                out=d0,
                in0=bass.AP(tensor=MINS.tensor, offset=MINS.offset,
                            ap=[MINS.ap[0], [4, NBH]]),
                in1=bass.AP(tensor=MINS.tensor, offset=MINS.offset + 1,
                            ap=[MINS.ap[0], [4, NBH]]),
                op=Alu.subtract,
            )
            d1 = state.tile([128, NBH], f32, tag="d1")
            nc.vector.tensor_tensor(
                out=d1,
                in0=bass.AP(tensor=MINS.tensor, offset=MINS.offset + 2,
                            ap=[MINS.ap[0], [4, NBH]]),
                in1=bass.AP(tensor=MINS.tensor, offset=MINS.offset + 3,
                            ap=[MINS.ap[0], [4, NBH]]),
                op=Alu.subtract,
            )
            mtsr = state.tile([128, NBH], f32, tag="mtsr")
            nc.vector.tensor_scalar(out=mtsr, in0=SM, scalar1=-1.0,
                                    scalar2=float(D), op0=Alu.mult, op1=Alu.add)
            r1 = state.tile([128, NBH], f32, tag="r1")
            nc.vector.tensor_tensor(out=r1, in0=d1, in1=mtsr, op=Alu.mult)

            fin2 = state.tile([128, 4], f32, tag="fin2")
            nc.vector.tensor_reduce(out=fin2[:, 0:1], in_=d0, axis=AX.X,
                                    op=Alu.add)
            nc.vector.tensor_reduce(out=fin2[:, 1:2], in_=r1, axis=AX.X,
                                    op=Alu.add)

            # total = 0.5/EPS*rec0 + 1/EPS*rec1 + reo - ent
            #         - (BETA/2) * (S*D*NBH + klc - kla - klb)
            acc = state.tile([128, 1], f32, tag="acc")
            tmp = state.tile([128, 1], f32, tag="tmp")
            nc.vector.tensor_scalar(out=acc, in0=fin2[:, 0:1],
                                    scalar1=0.5 / EPS, scalar2=None,
                                    op0=Alu.mult)
            nc.vector.scalar_tensor_tensor(
                out=acc, in0=fin2[:, 1:2], scalar=1.0 / EPS, in1=acc,
                op0=Alu.mult, op1=Alu.add,
            )
            nc.vector.tensor_tensor(out=acc, in0=acc, in1=fin[:, 4:5],
                                    op=Alu.add)
            nc.vector.tensor_tensor(out=acc, in0=acc, in1=fin[:, 3:4],
                                    op=Alu.subtract)
            nc.vector.tensor_scalar(out=tmp, in0=fin[:, 2:3],
                                    scalar1=float(S * D * NBH), scalar2=None,
                                    op0=Alu.add)
            nc.vector.tensor_tensor(out=tmp, in0=tmp, in1=fin[:, 0:1],
                                    op=Alu.subtract)
            nc.vector.tensor_tensor(out=tmp, in0=tmp, in1=fin[:, 1:2],
                                    op=Alu.subtract)
            nc.vector.scalar_tensor_tensor(
                out=acc, in0=tmp, scalar=-BETA / 2.0, in1=acc,
                op0=Alu.mult, op1=Alu.add,
            )

            pfin = ptot.tile([1, 1], f32, tag="tA", name="pfin")
            nc.tensor.matmul(pfin, acc, ones_f)
            osb = state.tile([1, 1], f32, tag="osb")
            nc.scalar.copy(out=osb, in_=pfin)
            nc.sync.dma_start(out=out_d, in_=osb)

    nc.compile()
    return nc


def _get_nc():
    if "nc" not in _nc_cache:
        _nc_cache["nc"] = build_bass()
    return _nc_cache["nc"]


def _prep_core(ai, ah, o, mu, lv, mask):
    f16 = np.float16
    ai_t = np.ascontiguousarray(ai.transpose(2, 0, 1).reshape(128, NBH, 128, 7),
                                dtype=f16)
    ah_t = np.ascontiguousarray(ah.transpose(2, 0, 1).reshape(128, NBH, 128, 7),
                                dtype=f16)
    o_t = np.ascontiguousarray(o.transpose(2, 0, 1).reshape(128, NBH, 128, 7),
                               dtype=f16)
    ah_n = np.ascontiguousarray(
        ah.reshape(NBH, 128, 896).transpose(1, 0, 2), dtype=f16)
    mu_n = np.ascontiguousarray(
        mu.reshape(NBH, 128, 896).transpose(1, 0, 2), dtype=f16)
    lv_n = np.ascontiguousarray(
        lv.reshape(NBH, 128, 896).transpose(1, 0, 2), dtype=f16)
    m = mask[:, 0, :]
    mask_n = np.ascontiguousarray(
        m.reshape(NBH, 128, 128).transpose(1, 0, 2), dtype=np.float32)
    mask_t = np.ascontiguousarray(
        m.T.reshape(128, NBH, 128), dtype=np.float32)
    return {"ai_t": ai_t, "ah_t": ah_t, "o_t": o_t, "ah_n": ah_n,
            "mu_n": mu_n, "lv_n": lv_n, "mask_n": mask_n, "mask_t": mask_t}


def kernel(ai, a_hat, mu_q, logvar_q, o, learned_mask, gamma):
    from concourse.bass_utils import run_bass_kernel_spmd

    nc = _get_nc()
    ai = np.asarray(ai, np.float32)
    a_hat = np.asarray(a_hat, np.float32)
    mu_q = np.asarray(mu_q, np.float32)
    logvar_q = np.asarray(logvar_q, np.float32)
    o = np.asarray(o, np.float32)
    learned_mask = np.asarray(learned_mask, np.float32)
    gam = np.ascontiguousarray(gamma, np.float32)

    in_maps = []
    for c in range(N_CORES):
        sl = slice(c * B, (c + 1) * B)
        m = _prep_core(ai[sl], a_hat[sl], o[sl], mu_q[sl], logvar_q[sl],
                       learned_mask[sl])
        m["gamma"] = gam
        in_maps.append(m)

    res = run_bass_kernel_spmd(
        nc, in_maps, core_ids=list(range(N_CORES)),
        trace=bool(int(os.environ.get("KBENCH_TRACE", "0"))),
    )
    total = np.float32(0.0)
    for r in res.results:
        total += np.float32(r["out"][0, 0])
    if res.exec_time_ns is not None:
        kernel.last_exec_time_ns = res.exec_time_ns
    kernel.last_results = res
    return np.asarray(total, dtype=np.float32)


kernel.last_exec_time_ns = None
kernel.last_results = None
